# revision 1
# baseline (speedup 1.0000x reference)
"""Graphormer-style dense transformer on 8 TRN2 NeuronCores.

Data-parallel: 2 graphs per core, zero collectives.
Feature-major residual stream => no on-device transposes anywhere.
"""
import sys

sys.path.insert(0, "/opt/trn_rl_repo")

import numpy as np
import ml_dtypes

import concourse.bass as bass
import concourse.mybir as mybir
import concourse.tile as tile
from concourse import bacc
from concourse.bass_utils import run_bass_kernel_spmd

F32 = mybir.dt.float32
BF16 = mybir.dt.bfloat16
AF = mybir.ActivationFunctionType
ALU = mybir.AluOpType

L, HEADS, IN_DIM, H, OUT, BD, FFN, G = 6, 12, 128, 768, 10, 8, 3072, 1
B, N = 16, 448
S = N + G            # 449
SP = 512             # padded sequence (k-side padding)
DK = H // HEADS      # 64
SCALE = DK ** -0.5
EPS = 1e-5
HT = H // 128        # 6
FT = FFN // 128      # 24
NCORES = 8
GPC = B // NCORES    # graphs per core = 2
NEG = -60.0  # exp(NEG) ~ 9e-27: kills padded keys, stays in ACT-LUT domain

_CACHE = {}


def build_nc(n_layers=L, gpc=GPC, taps=False):
    nc = bacc.Bacc("TRN2", target_bir_lowering=False, debug=False)

    # ---- DRAM parameters (per-core shard; host-prepped layouts) ----
    d_xT = nc.declare_dram_parameter("xT", [gpc, 128, S], BF16, isOutput=False)
    d_gtT = nc.declare_dram_parameter("gtT", [128, HT], F32, isOutput=False)
    d_encW = nc.declare_dram_parameter("encW", [128, H], BF16, isOutput=False)
    d_encB = nc.declare_dram_parameter("encB", [128, HT], F32, isOutput=False)
    d_wq = nc.declare_dram_parameter("wq", [L, H, H], BF16, isOutput=False)
    d_wk = nc.declare_dram_parameter("wk", [L, H, H], BF16, isOutput=False)
    d_wv = nc.declare_dram_parameter("wv", [L, H, H], BF16, isOutput=False)
    d_wo = nc.declare_dram_parameter("wo", [L, H, H], BF16, isOutput=False)
    d_bq = nc.declare_dram_parameter("bq", [L, 128, HT], F32, isOutput=False)
    d_bk = nc.declare_dram_parameter("bk", [L, 128, HT], F32, isOutput=False)
    d_bo = nc.declare_dram_parameter("bo", [L, 128, HT], F32, isOutput=False)
    d_bvb = nc.declare_dram_parameter("bvb", [L, 128, H], F32, isOutput=False)
    d_l1g = nc.declare_dram_parameter("l1g", [L, 128, HT], F32, isOutput=False)
    d_l1b = nc.declare_dram_parameter("l1b", [L, 128, HT], F32, isOutput=False)
    d_l2g = nc.declare_dram_parameter("l2g", [L, 128, HT], F32, isOutput=False)
    d_l2b = nc.declare_dram_parameter("l2b", [L, 128, HT], F32, isOutput=False)
    d_w1 = nc.declare_dram_parameter("w1", [L, H, FFN], BF16, isOutput=False)
    d_b1 = nc.declare_dram_parameter("b1", [L, 128, FT], F32, isOutput=False)
    d_w2 = nc.declare_dram_parameter("w2", [L, FFN, H], BF16, isOutput=False)
    d_b2 = nc.declare_dram_parameter("b2", [L, 128, HT], F32, isOutput=False)
    # attention bias, host-projected: [g, l, h, kp, kc, q] with k = kc*128+kp,
    # rows k>=449 filled with NEG
    d_bias = nc.declare_dram_parameter(
        "biasp", [gpc, L, HEADS, 128, 4, S], BF16, isOutput=False)
    d_hout = nc.declare_dram_parameter("hout", [gpc, HT, 128], F32, isOutput=True)

    tapd = {}
    if taps:
        for nm, shp in [("t_y", [128, HT, S]), ("t_q", [128, HT, S]),
                        ("t_k", [128, HT, SP]), ("t_e", [128, 4, S]),
                        ("t_o", [128, HT, S]), ("t_h1", [128, HT, S]),
                        ("t_u", [128, FT, S])]:
            tapd[nm] = nc.declare_dram_parameter(nm, shp, F32 if nm in ("t_h1",) else BF16, isOutput=True)

    with tile.TileContext(nc) as tc:
        # ---------------- pools ----------------
        const = tc.alloc_tile_pool(name="const", bufs=1)
        wqp = tc.alloc_tile_pool(name="wqp", bufs=2)
        wkp = tc.alloc_tile_pool(name="wkp", bufs=2)
        wvp = tc.alloc_tile_pool(name="wvp", bufs=1)
        wop = tc.alloc_tile_pool(name="wop", bufs=1)
        w1p = tc.alloc_tile_pool(name="w1p", bufs=3)
        w2p = tc.alloc_tile_pool(name="w2p", bufs=2)
        lcp = tc.alloc_tile_pool(name="lcp", bufs=2)   # per-layer small consts
        bip = tc.alloc_tile_pool(name="bip", bufs=2)   # bias stream
        ep = tc.alloc_tile_pool(name="ep", bufs=5)     # exp tiles
        wk_ = tc.alloc_tile_pool(name="wk_", bufs=2)   # f32 scratch [128,S]
        sqp = tc.alloc_tile_pool(name="sqp", bufs=1)   # square scratch
        stp = tc.alloc_tile_pool(name="stp", bufs=2)   # small stat rows
        xp = tc.alloc_tile_pool(name="xp", bufs=2)

        ps_misc = tc.alloc_tile_pool(name="ps_misc", bufs=2, space="PSUM")
        ps_proj = tc.alloc_tile_pool(name="ps_proj", bufs=2, space="PSUM")
        ps_st = tc.alloc_tile_pool(name="ps_st", bufs=2, space="PSUM")
        ps_o = tc.alloc_tile_pool(name="ps_o", bufs=2, space="PSUM")

        # ---------------- persistent tiles ----------------
        _frees = []

        def ptile(shape, dt, name):
            t, f = tc.tile(shape, dt, name=name)
            _frees.append(f)
            return t

        hT = [ptile([128, HT, S], F32, f"hT{g}") for g in range(gpc)]
        yT = ptile([128, HT, SP], BF16, "yT")
        qT = ptile([128, HT, S], BF16, "qT")
        kT = ptile([128, HT, SP], BF16, "kT")
        vaug = ptile([128, 4, HEADS, DK + 1], BF16, "vaug")
        oT = ptile([128, HT, S], BF16, "oT")
        uT = ptile([128, FT, S], BF16, "uT")

        ones_f = const.tile([128, 1], F32)
        nc.vector.memset(ones_f, 1.0)
        ones_b = const.tile([128, 1], BF16)
        nc.vector.memset(ones_b, 1.0)
        ones_row = const.tile([1, 128], F32)
        nc.vector.memset(ones_row, 1.0)
        eps_t = const.tile([1, 1], F32)
        nc.vector.memset(eps_t, EPS)

        # zero padding columns once; compute writes only [:, :, :S]
        nc.vector.memset(yT[:, :, S:SP], 0.0)
        nc.vector.memset(kT[:, :, S:SP], 0.0)
        nc.vector.memset(vaug[:, :, :, DK], 1.0)  # ones column -> softmax denom

        encw_sb = const.tile([128, H], BF16)
        nc.sync.dma_start(encw_sb, d_encW[:])
        encb_sb = const.tile([128, HT], F32)
        nc.sync.dma_start(encb_sb, d_encB[:])

        # ---------------- encoder ----------------
        for g in range(gpc):
            xt = xp.tile([128, S], BF16)
            nc.sync.dma_start(xt, d_xT[g])
            for t in range(HT):
                ps = ps_proj.tile([128, S], F32, tag="proj")
                nc.tensor.matmul(ps, encw_sb[:, t * 128:(t + 1) * 128], xt,
                                 start=True, stop=True)
                nc.scalar.activation(hT[g][:, t, :], ps, AF.Identity,
                                     bias=encb_sb[:, t:t + 1])
            # graph token column
            nc.sync.dma_start(hT[g][:, :, N], d_gtT[:])

        # ---------------- helpers ----------------
        def layer_norm(src, dst, g_sb, b_sb, gcol, bcol):
            """src: [128,HT,S] f32; dst: [128,HT,>=S] bf16 (writes [:, :, :S])."""
            sq = sqp.tile([128, HT, S], BF16, tag="sq")
            nc.scalar.activation(sq, src, AF.Square)
            sums = ps_misc.tile([128, S], F32, tag="mb", name="sums")[0:1, :]
            sumsq = ps_misc.tile([128, S], F32, tag="mb", name="sumsq")[0:1, :]
            for t in range(HT):
                nc.tensor.matmul(sums, ones_f, src[:, t, :],
                                 start=(t == 0), stop=(t == HT - 1))
            for t in range(HT):
                nc.tensor.matmul(sumsq, ones_b, sq[:, t, :],
                                 start=(t == 0), stop=(t == HT - 1))
            mean = stp.tile([1, S], F32, tag="srow")
            nc.vector.tensor_scalar_mul(mean, sums, 1.0 / H)
            var = stp.tile([1, S], F32, tag="srow")
            # var = sumsq/H - mean^2
            m2 = stp.tile([1, S], F32, tag="srow")
            nc.vector.tensor_mul(m2, mean, mean)
            nc.vector.tensor_scalar(var, sumsq, 1.0 / H, None, ALU.mult)
            nc.vector.tensor_sub(var, var, m2)
            std = stp.tile([1, S], F32, tag="srow")
            nc.scalar.activation(std, var, AF.Sqrt, bias=eps_t)
            rstd = stp.tile([1, S], F32, tag="srow")
            nc.vector.reciprocal(rstd, std)
            mb_ps = ps_misc.tile([128, S], F32, tag="mb", name="mb_ps")
            nc.tensor.matmul(mb_ps, ones_row, mean, start=True, stop=True)
            mean_bc = wk_.tile([128, S], F32, tag="wbc")
            nc.scalar.copy(mean_bc, mb_ps)
            rb_ps = ps_misc.tile([128, S], F32, tag="mb", name="rb_ps")
            nc.tensor.matmul(rb_ps, ones_row, rstd, start=True, stop=True)
            rstd_bc = wk_.tile([128, S], F32, tag="wbc")
            nc.scalar.copy(rstd_bc, rb_ps)
            for t in range(HT):
                tmp = wk_.tile([128, S], F32, tag="wtmp")
                nc.vector.tensor_sub(tmp, src[:, t, :], mean_bc)
                nc.vector.tensor_mul(tmp, tmp, rstd_bc)
                nc.vector.tensor_scalar(dst[:, t, :S], tmp,
                                        g_sb[:, gcol + t:gcol + t + 1],
                                        b_sb[:, bcol + t:bcol + t + 1],
                                        ALU.mult, ALU.add)

        # ---------------- layers ----------------
        for l in range(n_layers):
            wq_sb = wqp.tile([128, HT, H], BF16, tag="wq")
            nc.sync.dma_start(wq_sb, d_wq[l].rearrange("(kt kp) m -> kp kt m", kp=128))
            wk_sb = wkp.tile([128, HT, H], BF16, tag="wk")
            nc.sync.dma_start(wk_sb, d_wk[l].rearrange("(kt kp) m -> kp kt m", kp=128))
            wv_sb = wvp.tile([128, HT, H], BF16, tag="wv")
            nc.sync.dma_start(wv_sb, d_wv[l].rearrange("(kt kp) m -> kp kt m", kp=128))
            wo_sb = wop.tile([128, HT, H], BF16, tag="wo")
            nc.sync.dma_start(wo_sb, d_wo[l].rearrange("(kt kp) m -> kp kt m", kp=128))
            lc = lcp.tile([128, 6 * HT + FT], F32, tag="lc")
            nc.sync.dma_start(lc[:, 0:HT], d_bq[l])
            nc.sync.dma_start(lc[:, HT:2 * HT], d_bk[l])
            nc.sync.dma_start(lc[:, 2 * HT:3 * HT], d_bo[l])
            nc.sync.dma_start(lc[:, 3 * HT:4 * HT], d_l1g[l])
            nc.sync.dma_start(lc[:, 4 * HT:5 * HT], d_l1b[l])
            nc.sync.dma_start(lc[:, 5 * HT:6 * HT], d_l2g[l])
            nc.sync.dma_start(lc[:, 6 * HT:6 * HT + FT], d_b1[l])
            lc2 = lcp.tile([128, HT + H], F32, tag="lc2")
            nc.sync.dma_start(lc2[:, 0:HT], d_b2[l])
            nc.sync.dma_start(lc2[:, HT:HT + H], d_bvb[l])
            l2b_sb = lcp.tile([128, HT], F32, tag="l2b")
            nc.sync.dma_start(l2b_sb, d_l2b[l])

            for g in range(gpc):
                # ---- LN1 ----
                layer_norm(hT[g], yT, lc, lc, 3 * HT, 4 * HT)
                if taps and l == 0 and g == 0:
                    nc.sync.dma_start(tapd["t_y"][:], yT[:, :, :S])

                # ---- q, k projections (feature-major) ----
                for dst, w_sb, bcol in ((qT, wq_sb, 0), (kT, wk_sb, HT)):
                    for m in range(HT):
                        ps = ps_proj.tile([128, S], F32, tag="proj")
                        for kt in range(HT):
                            nc.tensor.matmul(ps, w_sb[:, kt, m * 128:(m + 1) * 128],
                                             yT[:, kt, :S],
                                             start=(kt == 0), stop=(kt == HT - 1))
                        nc.vector.tensor_scalar(dst[:, m, :S], ps,
                                                lc[:, bcol + m:bcol + m + 1], None,
                                                ALU.add)
                # ---- v projection (row-major, ones-augmented) ----
                for c in range(4):
                    for j in range(2):
                        ps = ps_proj.tile([128, 384], F32, tag="proj")
                        for kt in range(HT):
                            nc.tensor.matmul(ps, yT[:, kt, c * 128:(c + 1) * 128],
                                             wv_sb[:, kt, j * 384:(j + 1) * 384],
                                             start=(kt == 0), stop=(kt == HT - 1))
                        nc.vector.tensor_tensor(
                            vaug[:, c, 6 * j:6 * (j + 1), 0:DK],
                            ps.rearrange("p (h d) -> p h d", d=DK),
                            lc2[:, HT + j * 384:HT + (j + 1) * 384].rearrange(
                                "p (h d) -> p h d", d=DK),
                            ALU.add)
                if taps and l == 0 and g == 0:
                    nc.sync.dma_start(tapd["t_q"][:], qT[:])
                    nc.sync.dma_start(tapd["t_k"][:], kT[:])

                # ---- attention, head pairs ----
                for t in range(HT):
                    ot_ps = []
                    rden_p = [stp.tile([1, S], F32, tag="rdn", name="rdn0"),
                              stp.tile([1, S], F32, tag="rdn2", name="rdn1")]
                    for j in range(2):
                        hh = 2 * t + j
                        r = j * DK
                        e_tiles = []
                        bias_sb = bip.tile([128, 4, S], BF16, tag="bias")
                        nc.sync.dma_start(bias_sb, d_bias[g, l, hh])
                        for c in range(4):
                            st = ps_st.tile([128, S], F32, tag="st")
                            nc.tensor.matmul(st, kT[r:r + DK, t, c * 128:(c + 1) * 128],
                                             qT[r:r + DK, t, :S], start=True, stop=True)
                            nc.vector.tensor_tensor(st, st, bias_sb[:, c, :], ALU.add)
                            e = ep.tile([128, S], BF16, tag="e")
                            nc.scalar.activation(e, st, AF.Exp)
                            e_tiles.append(e)
                        if taps and l == 0 and g == 0 and hh == 0:
                            for c in range(4):
                                nc.sync.dma_start(tapd["t_e"][:, c, :], e_tiles[c])
                        op = ps_o.tile([DK + 1, S], F32, tag="ot")
                        for c in range(4):
                            nc.tensor.matmul(op, vaug[:, c, hh, :], e_tiles[c],
                                             start=(c == 0), stop=(c == 3))
                        nc.vector.reciprocal(rden_p[j], op[DK:DK + 1, :])
                        ot_ps.append(op)
                    bc_ps = ps_misc.tile([128, S], F32, tag="mb", name="bc_ps")
                    nc.tensor.matmul(bc_ps[0:64, :], ones_row[:, 0:64],
                                     rden_p[0], start=True, stop=True)
                    nc.tensor.matmul(bc_ps[64:128, :], ones_row[:, 0:64],
                                     rden_p[1], start=True, stop=True)
                    bc_sb = wk_.tile([128, S], F32, tag="wbc")
                    nc.scalar.copy(bc_sb, bc_ps)
                    for j in range(2):
                        nc.vector.tensor_mul(oT[j * DK:(j + 1) * DK, t, :S],
                                             ot_ps[j][0:DK, :],
                                             bc_sb[j * DK:(j + 1) * DK, :])
                if taps and l == 0 and g == 0:
                    nc.sync.dma_start(tapd["t_o"][:], oT[:])

                # ---- output projection + residual ----
                for m in range(HT):
                    ps = ps_proj.tile([128, S], F32, tag="proj")
                    for kt in range(HT):
                        nc.tensor.matmul(ps, wo_sb[:, kt, m * 128:(m + 1) * 128],
                                         oT[:, kt, :S],
                                         start=(kt == 0), stop=(kt == HT - 1))
                    nc.scalar.activation(ps, ps, AF.Identity,
                                         bias=lc[:, 2 * HT + m:2 * HT + m + 1])
                    nc.vector.tensor_add(hT[g][:, m, :], hT[g][:, m, :], ps)
                if taps and l == 0 and g == 0:
                    nc.sync.dma_start(tapd["t_h1"][:], hT[g][:])

                # ---- FFN ----
                layer_norm(hT[g], yT, lc, l2b_sb, 5 * HT, 0)
                for m in range(FT):
                    w1t = w1p.tile([128, HT, 128], BF16, tag="w1")
                    nc.sync.dma_start(
                        w1t, d_w1[l].rearrange("(kt kp) f -> kp kt f",
                                               kp=128)[:, :, m * 128:(m + 1) * 128])
                    ps = ps_proj.tile([128, S], F32, tag="proj")
                    for kt in range(HT):
                        nc.tensor.matmul(ps, w1t[:, kt, :], yT[:, kt, :S],
                                         start=(kt == 0), stop=(kt == HT - 1))
                    nc.scalar.activation(uT[:, m, :S], ps, AF.Gelu,
                                         bias=lc[:, 6 * HT + m:6 * HT + m + 1])
                if taps and l == 0 and g == 0:
                    nc.sync.dma_start(tapd["t_u"][:], uT[:])
                for m in range(HT):
                    w2t = w2p.tile([128, FT, 128], BF16, tag="w2")
                    nc.sync.dma_start(
                        w2t, d_w2[l].rearrange("(kt kp) m -> kp kt m",
                                               kp=128)[:, :, m * 128:(m + 1) * 128])
                    ps = ps_proj.tile([128, S], F32, tag="proj")
                    for kt in range(FT):
                        nc.tensor.matmul(ps, w2t[:, kt, :], uT[:, kt, :S],
                                         start=(kt == 0), stop=(kt == FT - 1))
                    nc.scalar.activation(ps, ps, AF.Identity,
                                         bias=lc2[:, m:m + 1])
                    nc.vector.tensor_add(hT[g][:, m, :], hT[g][:, m, :], ps)

        # ---------------- output: h[:, 0, :] per graph ----------------
        for g in range(gpc):
            nc.sync.dma_start(d_hout[g].rearrange("t p -> p t"), hT[g][:, :, 0])

        # close pools in LIFO order: singles first, then named pools reversed
        for f in reversed(_frees):
            f()
        for p in (ps_o, ps_st, ps_proj, ps_misc, xp, stp, sqp, wk_, ep, bip,
                  lcp, w2p, w1p, wop, wvp, wkp, wqp, const):
            p.release()

    nc.compile()
    return nc


# ================= host side =================

def _bf16(a):
    return np.asarray(a, np.float32).astype(ml_dtypes.bfloat16)


def _pcol(v):
    """[..., H-like] -> [..., 128, ntiles] per-partition layout."""
    nt = v.shape[-1] // 128
    return np.ascontiguousarray(
        v.reshape(v.shape[:-1] + (nt, 128)).swapaxes(-1, -2)).astype(np.float32)


def prep_inputs(inp, n_layers=L, gpc=GPC):
    """Full inputs dict -> list of per-core in_maps."""
    x = np.asarray(inp["x"], np.float32)
    attn_bias = np.asarray(inp["attn_bias"], np.float32)
    gvd = np.asarray(inp["gvd"], np.float32)
    Wb = np.asarray(inp["Wbias"], np.float32)      # [L, BD, HEADS]
    bbias = np.asarray(inp["bbias"], np.float32)   # [L, HEADS]

    # xT padded (col 448 = 0)
    xT = np.zeros((B, 128, S), np.float32)
    xT[:, :, :N] = x.transpose(0, 2, 1)
    xT = _bf16(xT)

    gtT = _pcol(np.asarray(inp["graph_token"], np.float32)[0][None])[0]  # [128, HT]
    encW = _bf16(inp["enc_W"])
    encB = _pcol(np.asarray(inp["enc_b"], np.float32)[None])[0]

    wq = _bf16(np.asarray(inp["Wq"], np.float32) * SCALE)
    wk = _bf16(inp["Wk"])
    wv = _bf16(inp["Wv"])
    wo = _bf16(inp["Wo"])
    bq = _pcol(np.asarray(inp["bq"], np.float32) * SCALE)
    bk = _pcol(np.asarray(inp["bk"], np.float32))
    bo = _pcol(np.asarray(inp["bo"], np.float32))
    bvb = np.ascontiguousarray(np.broadcast_to(
        np.asarray(inp["bv"], np.float32)[:, None, :], (L, 128, H))).astype(np.float32)
    l1g = _pcol(np.asarray(inp["ln1_g"], np.float32))
    l1b = _pcol(np.asarray(inp["ln1_b"], np.float32))
    l2g = _pcol(np.asarray(inp["ln2_g"], np.float32))
    l2b = _pcol(np.asarray(inp["ln2_b"], np.float32))
    w1 = _bf16(inp["W1"])
    b1 = _pcol(np.asarray(inp["b1"], np.float32))
    w2 = _bf16(inp["W2"])
    b2 = _pcol(np.asarray(inp["b2"], np.float32))

    # ---- attention bias projection on host ----
    # gb[b, q, k, c]; bias[l, h, q, k] = sum_c gb[q, k, c] W[l, c, h] + bbias
    # device wants [g, l, h, kp, kc, q] (k = kc*128 + kp), pad k>=S with NEG
    Wall = Wb.transpose(1, 0, 2).reshape(BD, L * HEADS)   # [c, l*h]
    biasp = np.empty((B, L, HEADS, 128, 4, S), ml_dtypes.bfloat16)
    gvd0 = gvd[0]
    for b in range(B):
        gb = np.empty((S, S, BD), np.float32)
        gb[:N, :N] = attn_bias[b]
        gb[N, :N] = gvd0
        gb[:, N] = gvd0
        pr = gb.reshape(S * S, BD) @ Wall                 # [(q k), l*h]
        pr = pr.reshape(S, S, L, HEADS) + bbias[None, None]
        # -> [l, h, k, q], pad k to 512
        pr = pr.transpose(2, 3, 1, 0)
        prp = np.full((L, HEADS, SP, S), NEG, np.float32)
        prp[:, :, :S, :] = pr
        biasp[b] = _bf16(prp.reshape(L, HEADS, 4, 128, S).transpose(0, 1, 3, 2, 4))

    shared = dict(gtT=gtT, encW=encW, encB=encB, wq=wq, wk=wk, wv=wv, wo=wo,
                  bq=bq, bk=bk, bo=bo, bvb=bvb, l1g=l1g, l1b=l1b, l2g=l2g,
                  l2b=l2b, w1=w1, b1=b1, w2=w2, b2=b2)
    in_maps = []
    for core in range(B // gpc):
        m = dict(shared)
        m["xT"] = np.ascontiguousarray(xT[core * gpc:(core + 1) * gpc])
        m["biasp"] = np.ascontiguousarray(biasp[core * gpc:(core + 1) * gpc])
        in_maps.append(m)
    return in_maps


def finish_host(h0, inp):
    """h0: [B, H] final residual at node 0 (pre final-LN). -> [B, OUT] log_softmax."""
    fg = np.asarray(inp["fln_g"], np.float32)
    fb = np.asarray(inp["fln_b"], np.float32)
    oW = np.asarray(inp["out_W"], np.float32)
    ob = np.asarray(inp["out_b"], np.float32)
    m = h0.mean(-1, keepdims=True)
    v = np.square(h0 - m).mean(-1, keepdims=True)
    y = (h0 - m) / np.sqrt(v + EPS) * fg + fb
    logits = y @ oW + ob
    z = logits - logits.max(-1, keepdims=True)
    return (z - np.log(np.exp(z).sum(-1, keepdims=True))).astype(np.float32)


def kernel(**inputs):
    if "nc" not in _CACHE:
        _CACHE["nc"] = build_nc()
    nc = _CACHE["nc"]
    in_maps = prep_inputs(inputs)
    res = run_bass_kernel_spmd(nc, in_maps, core_ids=list(range(NCORES)))
    _CACHE["exec_time_ns"] = getattr(res, "exec_time_ns", None)
    h0 = np.concatenate([r["hout"].reshape(GPC, H) for r in res.results], axis=0)
    return finish_host(h0, inputs)


if __name__ == "__main__":
    import reference
    inp = reference.setup_inputs()
    inp = {k: np.asarray(v) for k, v in inp.items()}
    out = kernel(**inp)
    exp = np.asarray(reference.reference(**inp))
    err = np.abs(out - exp).max() / np.abs(exp).max()
    print("Relative error:", err)



# revision 2
# speedup vs baseline: 14128.6886x; 14128.6886x over previous
"""Graphormer-style dense transformer on 8 TRN2 NeuronCores.

Data-parallel: 2 graphs per core, zero collectives.
Feature-major residual stream => no on-device transposes anywhere.
"""
import sys

sys.path.insert(0, "/opt/trn_rl_repo")

import numpy as np
import ml_dtypes

import concourse.bass as bass
import concourse.mybir as mybir
import concourse.tile as tile
from concourse import bacc
from concourse.bass_utils import run_bass_kernel_spmd

F32 = mybir.dt.float32
BF16 = mybir.dt.bfloat16
AF = mybir.ActivationFunctionType
ALU = mybir.AluOpType

L, HEADS, IN_DIM, H, OUT, BD, FFN, G = 6, 12, 128, 768, 10, 8, 3072, 1
B, N = 16, 448
S = N + G            # 449
SP = 512             # padded sequence (k-side padding)
DK = H // HEADS      # 64
SCALE = DK ** -0.5
EPS = 1e-5
HT = H // 128        # 6
FT = FFN // 128      # 24
NCORES = 8
GPC = B // NCORES    # graphs per core = 2
NEG = -60.0  # exp(NEG) ~ 9e-27: kills padded keys, stays in ACT-LUT domain

_CACHE = {}


def build_nc(n_layers=L, gpc=GPC, taps=False):
    nc = bacc.Bacc("TRN2", target_bir_lowering=False, debug=False)

    # ---- DRAM parameters (per-core shard; host-prepped layouts) ----
    d_xT = nc.declare_dram_parameter("xT", [gpc, 128, S], BF16, isOutput=False)
    d_gtT = nc.declare_dram_parameter("gtT", [128, HT], F32, isOutput=False)
    d_encW = nc.declare_dram_parameter("encW", [128, H], BF16, isOutput=False)
    d_encB = nc.declare_dram_parameter("encB", [128, HT], F32, isOutput=False)
    d_wq = nc.declare_dram_parameter("wq", [L, H, H], BF16, isOutput=False)
    d_wk = nc.declare_dram_parameter("wk", [L, H, H], BF16, isOutput=False)
    d_wv = nc.declare_dram_parameter("wv", [L, H, H], BF16, isOutput=False)
    d_wo = nc.declare_dram_parameter("wo", [L, H, H], BF16, isOutput=False)
    d_bq = nc.declare_dram_parameter("bq", [L, 128, HT], F32, isOutput=False)
    d_bk = nc.declare_dram_parameter("bk", [L, 128, HT], F32, isOutput=False)
    d_bo = nc.declare_dram_parameter("bo", [L, 128, HT], F32, isOutput=False)
    d_bvb = nc.declare_dram_parameter("bvb", [L, 128, H], F32, isOutput=False)
    d_l1g = nc.declare_dram_parameter("l1g", [L, 128, HT], F32, isOutput=False)
    d_l1b = nc.declare_dram_parameter("l1b", [L, 128, HT], F32, isOutput=False)
    d_l2g = nc.declare_dram_parameter("l2g", [L, 128, HT], F32, isOutput=False)
    d_l2b = nc.declare_dram_parameter("l2b", [L, 128, HT], F32, isOutput=False)
    d_w1 = nc.declare_dram_parameter("w1", [L, H, FFN], BF16, isOutput=False)
    d_b1 = nc.declare_dram_parameter("b1", [L, 128, FT], F32, isOutput=False)
    d_w2 = nc.declare_dram_parameter("w2", [L, FFN, H], BF16, isOutput=False)
    d_b2 = nc.declare_dram_parameter("b2", [L, 128, HT], F32, isOutput=False)
    # attention bias, host-projected: [g, l, h, kp, kc, q] with k = kc*128+kp,
    # rows k>=449 filled with NEG
    d_bias = nc.declare_dram_parameter(
        "biasp", [gpc, L, HEADS, 128, 4, S], BF16, isOutput=False)
    d_hout = nc.declare_dram_parameter("hout", [gpc, HT, 128], F32, isOutput=True)

    tapd = {}
    if taps:
        for nm, shp in [("t_y", [128, HT, S]), ("t_q", [128, HT, S]),
                        ("t_k", [128, HT, SP]), ("t_e", [128, 4, S]),
                        ("t_o", [128, HT, S]), ("t_h1", [128, HT, S]),
                        ("t_u", [128, FT, S])]:
            tapd[nm] = nc.declare_dram_parameter(nm, shp, F32 if nm in ("t_h1",) else BF16, isOutput=True)

    with tile.TileContext(nc) as tc:
        # ---------------- pools ----------------
        const = tc.alloc_tile_pool(name="const", bufs=1)
        wqp = tc.alloc_tile_pool(name="wqp", bufs=2)
        wkp = tc.alloc_tile_pool(name="wkp", bufs=2)
        wvp = tc.alloc_tile_pool(name="wvp", bufs=1)
        wop = tc.alloc_tile_pool(name="wop", bufs=1)
        w1p = tc.alloc_tile_pool(name="w1p", bufs=3)
        w2p = tc.alloc_tile_pool(name="w2p", bufs=2)
        lcp = tc.alloc_tile_pool(name="lcp", bufs=2)   # per-layer small consts
        bip = tc.alloc_tile_pool(name="bip", bufs=2)   # bias stream
        ep = tc.alloc_tile_pool(name="ep", bufs=5)     # exp tiles
        wk_ = tc.alloc_tile_pool(name="wk_", bufs=2)   # f32 scratch [128,S]
        sqp = tc.alloc_tile_pool(name="sqp", bufs=1)   # square scratch
        stp = tc.alloc_tile_pool(name="stp", bufs=2)   # small stat rows
        xp = tc.alloc_tile_pool(name="xp", bufs=2)

        ps_misc = tc.alloc_tile_pool(name="ps_misc", bufs=2, space="PSUM")
        ps_proj = tc.alloc_tile_pool(name="ps_proj", bufs=2, space="PSUM")
        ps_st = tc.alloc_tile_pool(name="ps_st", bufs=2, space="PSUM")
        ps_o = tc.alloc_tile_pool(name="ps_o", bufs=2, space="PSUM")

        # ---------------- persistent tiles ----------------
        _frees = []

        def ptile(shape, dt, name):
            t, f = tc.tile(shape, dt, name=name)
            _frees.append(f)
            return t

        hT = [ptile([128, HT, S], F32, f"hT{g}") for g in range(gpc)]
        yT = ptile([128, HT, SP], BF16, "yT")
        qT = ptile([128, HT, S], BF16, "qT")
        kT = ptile([128, HT, SP], BF16, "kT")
        vaug = ptile([128, 4, HEADS, DK + 1], BF16, "vaug")
        oT = ptile([128, HT, S], BF16, "oT")
        uT = ptile([128, FT, S], BF16, "uT")

        ones_f = const.tile([128, 1], F32)
        nc.vector.memset(ones_f, 1.0)
        ones_b = const.tile([128, 1], BF16)
        nc.vector.memset(ones_b, 1.0)
        ones_row = const.tile([1, 128], F32)
        nc.vector.memset(ones_row, 1.0)
        eps_t = const.tile([1, 1], F32)
        nc.vector.memset(eps_t, EPS)

        # zero padding columns once; compute writes only [:, :, :S]
        nc.vector.memset(yT[:, :, S:SP], 0.0)
        nc.vector.memset(kT[:, :, S:SP], 0.0)
        nc.vector.memset(vaug[:, :, :, DK], 1.0)  # ones column -> softmax denom

        encw_sb = const.tile([128, H], BF16)
        nc.sync.dma_start(encw_sb, d_encW[:])
        encb_sb = const.tile([128, HT], F32)
        nc.sync.dma_start(encb_sb, d_encB[:])

        # ---------------- encoder ----------------
        for g in range(gpc):
            xt = xp.tile([128, S], BF16)
            nc.sync.dma_start(xt, d_xT[g])
            for t in range(HT):
                ps = ps_proj.tile([128, S], F32, tag="proj")
                nc.tensor.matmul(ps, encw_sb[:, t * 128:(t + 1) * 128], xt,
                                 start=True, stop=True)
                nc.scalar.activation(hT[g][:, t, :], ps, AF.Identity,
                                     bias=encb_sb[:, t:t + 1])
            # graph token column
            nc.sync.dma_start(hT[g][:, :, N], d_gtT[:])

        # ---------------- helpers ----------------
        def layer_norm(src, dst, g_sb, b_sb, gcol, bcol):
            """src: [128,HT,S] f32; dst: [128,HT,>=S] bf16 (writes [:, :, :S])."""
            sq = sqp.tile([128, HT, S], BF16, tag="sq")
            nc.scalar.activation(sq, src, AF.Square)
            sums = ps_misc.tile([128, S], F32, tag="mb", name="sums")[0:1, :]
            sumsq = ps_misc.tile([128, S], F32, tag="mb", name="sumsq")[0:1, :]
            for t in range(HT):
                nc.tensor.matmul(sums, ones_f, src[:, t, :],
                                 start=(t == 0), stop=(t == HT - 1))
            for t in range(HT):
                nc.tensor.matmul(sumsq, ones_b, sq[:, t, :],
                                 start=(t == 0), stop=(t == HT - 1))
            mean = stp.tile([1, S], F32, tag="srow")
            nc.vector.tensor_scalar_mul(mean, sums, 1.0 / H)
            var = stp.tile([1, S], F32, tag="srow")
            # var = sumsq/H - mean^2
            m2 = stp.tile([1, S], F32, tag="srow")
            nc.vector.tensor_mul(m2, mean, mean)
            nc.vector.tensor_scalar(var, sumsq, 1.0 / H, None, ALU.mult)
            nc.vector.tensor_sub(var, var, m2)
            std = stp.tile([1, S], F32, tag="srow")
            nc.scalar.activation(std, var, AF.Sqrt, bias=eps_t)
            rstd = stp.tile([1, S], F32, tag="srow")
            nc.vector.reciprocal(rstd, std)
            mb_ps = ps_misc.tile([128, S], F32, tag="mb", name="mb_ps")
            nc.tensor.matmul(mb_ps, ones_row, mean, start=True, stop=True)
            mean_bc = wk_.tile([128, S], F32, tag="wbc")
            nc.scalar.copy(mean_bc, mb_ps)
            rb_ps = ps_misc.tile([128, S], F32, tag="mb", name="rb_ps")
            nc.tensor.matmul(rb_ps, ones_row, rstd, start=True, stop=True)
            rstd_bc = wk_.tile([128, S], F32, tag="wbc")
            nc.scalar.copy(rstd_bc, rb_ps)
            for t in range(HT):
                tmp = wk_.tile([128, S], F32, tag="wtmp")
                nc.vector.tensor_sub(tmp, src[:, t, :], mean_bc)
                nc.vector.tensor_mul(tmp, tmp, rstd_bc)
                nc.vector.tensor_scalar(dst[:, t, :S], tmp,
                                        g_sb[:, gcol + t:gcol + t + 1],
                                        b_sb[:, bcol + t:bcol + t + 1],
                                        ALU.mult, ALU.add)

        # ---------------- layers ----------------
        for l in range(n_layers):
            wq_sb = wqp.tile([128, HT, H], BF16, tag="wq")
            nc.sync.dma_start(wq_sb, d_wq[l].rearrange("(kt kp) m -> kp kt m", kp=128))
            wk_sb = wkp.tile([128, HT, H], BF16, tag="wk")
            nc.sync.dma_start(wk_sb, d_wk[l].rearrange("(kt kp) m -> kp kt m", kp=128))
            wv_sb = wvp.tile([128, HT, H], BF16, tag="wv")
            nc.sync.dma_start(wv_sb, d_wv[l].rearrange("(kt kp) m -> kp kt m", kp=128))
            wo_sb = wop.tile([128, HT, H], BF16, tag="wo")
            nc.sync.dma_start(wo_sb, d_wo[l].rearrange("(kt kp) m -> kp kt m", kp=128))
            lc = lcp.tile([128, 6 * HT + FT], F32, tag="lc")
            nc.sync.dma_start(lc[:, 0:HT], d_bq[l])
            nc.sync.dma_start(lc[:, HT:2 * HT], d_bk[l])
            nc.sync.dma_start(lc[:, 2 * HT:3 * HT], d_bo[l])
            nc.sync.dma_start(lc[:, 3 * HT:4 * HT], d_l1g[l])
            nc.sync.dma_start(lc[:, 4 * HT:5 * HT], d_l1b[l])
            nc.sync.dma_start(lc[:, 5 * HT:6 * HT], d_l2g[l])
            nc.sync.dma_start(lc[:, 6 * HT:6 * HT + FT], d_b1[l])
            lc2 = lcp.tile([128, HT + H], F32, tag="lc2")
            nc.sync.dma_start(lc2[:, 0:HT], d_b2[l])
            nc.sync.dma_start(lc2[:, HT:HT + H], d_bvb[l])
            l2b_sb = lcp.tile([128, HT], F32, tag="l2b")
            nc.sync.dma_start(l2b_sb, d_l2b[l])

            for g in range(gpc):
                # ---- LN1 ----
                layer_norm(hT[g], yT, lc, lc, 3 * HT, 4 * HT)
                if taps and l == 0 and g == 0:
                    nc.sync.dma_start(tapd["t_y"][:], yT[:, :, :S])

                # ---- q, k projections (feature-major) ----
                for dst, w_sb, bcol in ((qT, wq_sb, 0), (kT, wk_sb, HT)):
                    for m in range(HT):
                        ps = ps_proj.tile([128, S], F32, tag="proj")
                        for kt in range(HT):
                            nc.tensor.matmul(ps, w_sb[:, kt, m * 128:(m + 1) * 128],
                                             yT[:, kt, :S],
                                             start=(kt == 0), stop=(kt == HT - 1))
                        nc.vector.tensor_scalar(dst[:, m, :S], ps,
                                                lc[:, bcol + m:bcol + m + 1], None,
                                                ALU.add)
                # ---- v projection (row-major, ones-augmented) ----
                for c in range(4):
                    for j in range(2):
                        ps = ps_proj.tile([128, 384], F32, tag="proj")
                        for kt in range(HT):
                            nc.tensor.matmul(ps, yT[:, kt, c * 128:(c + 1) * 128],
                                             wv_sb[:, kt, j * 384:(j + 1) * 384],
                                             start=(kt == 0), stop=(kt == HT - 1))
                        nc.vector.tensor_tensor(
                            vaug[:, c, 6 * j:6 * (j + 1), 0:DK],
                            ps.rearrange("p (h d) -> p h d", d=DK),
                            lc2[:, HT + j * 384:HT + (j + 1) * 384].rearrange(
                                "p (h d) -> p h d", d=DK),
                            ALU.add)
                if taps and l == 0 and g == 0:
                    nc.sync.dma_start(tapd["t_q"][:], qT[:])
                    nc.sync.dma_start(tapd["t_k"][:], kT[:])

                # ---- attention, head pairs ----
                for t in range(HT):
                    ot_ps = []
                    rden_p = [stp.tile([1, S], F32, tag="rdn", name="rdn0"),
                              stp.tile([1, S], F32, tag="rdn2", name="rdn1")]
                    for j in range(2):
                        hh = 2 * t + j
                        r = j * DK
                        e_tiles = []
                        bias_sb = bip.tile([128, 4, S], BF16, tag="bias")
                        nc.sync.dma_start(bias_sb, d_bias[g, l, hh])
                        for c in range(4):
                            st = ps_st.tile([128, S], F32, tag="st")
                            nc.tensor.matmul(st, kT[r:r + DK, t, c * 128:(c + 1) * 128],
                                             qT[r:r + DK, t, :S], start=True, stop=True)
                            nc.vector.tensor_tensor(st, st, bias_sb[:, c, :], ALU.add)
                            e = ep.tile([128, S], BF16, tag="e")
                            nc.scalar.activation(e, st, AF.Exp)
                            e_tiles.append(e)
                        if taps and l == 0 and g == 0 and hh == 0:
                            for c in range(4):
                                nc.sync.dma_start(tapd["t_e"][:, c, :], e_tiles[c])
                        op = ps_o.tile([DK + 1, S], F32, tag="ot")
                        for c in range(4):
                            nc.tensor.matmul(op, vaug[:, c, hh, :], e_tiles[c],
                                             start=(c == 0), stop=(c == 3))
                        nc.vector.reciprocal(rden_p[j], op[DK:DK + 1, :])
                        ot_ps.append(op)
                    bc_ps = ps_misc.tile([128, S], F32, tag="mb", name="bc_ps")
                    nc.tensor.matmul(bc_ps[0:64, :], ones_row[:, 0:64],
                                     rden_p[0], start=True, stop=True)
                    nc.tensor.matmul(bc_ps[64:128, :], ones_row[:, 0:64],
                                     rden_p[1], start=True, stop=True)
                    bc_sb = wk_.tile([128, S], F32, tag="wbc")
                    nc.scalar.copy(bc_sb, bc_ps)
                    for j in range(2):
                        nc.vector.tensor_mul(oT[j * DK:(j + 1) * DK, t, :S],
                                             ot_ps[j][0:DK, :],
                                             bc_sb[j * DK:(j + 1) * DK, :])
                if taps and l == 0 and g == 0:
                    nc.sync.dma_start(tapd["t_o"][:], oT[:])

                # ---- output projection + residual ----
                for m in range(HT):
                    ps = ps_proj.tile([128, S], F32, tag="proj")
                    for kt in range(HT):
                        nc.tensor.matmul(ps, wo_sb[:, kt, m * 128:(m + 1) * 128],
                                         oT[:, kt, :S],
                                         start=(kt == 0), stop=(kt == HT - 1))
                    nc.scalar.activation(ps, ps, AF.Identity,
                                         bias=lc[:, 2 * HT + m:2 * HT + m + 1])
                    nc.vector.tensor_add(hT[g][:, m, :], hT[g][:, m, :], ps)
                if taps and l == 0 and g == 0:
                    nc.sync.dma_start(tapd["t_h1"][:], hT[g][:])

                # ---- FFN ----
                layer_norm(hT[g], yT, lc, l2b_sb, 5 * HT, 0)
                for m in range(FT):
                    w1t = w1p.tile([128, HT, 128], BF16, tag="w1")
                    nc.sync.dma_start(
                        w1t, d_w1[l].rearrange("(kt kp) f -> kp kt f",
                                               kp=128)[:, :, m * 128:(m + 1) * 128])
                    ps = ps_proj.tile([128, S], F32, tag="proj")
                    for kt in range(HT):
                        nc.tensor.matmul(ps, w1t[:, kt, :], yT[:, kt, :S],
                                         start=(kt == 0), stop=(kt == HT - 1))
                    nc.scalar.activation(uT[:, m, :S], ps, AF.Gelu,
                                         bias=lc[:, 6 * HT + m:6 * HT + m + 1])
                if taps and l == 0 and g == 0:
                    nc.sync.dma_start(tapd["t_u"][:], uT[:])
                for m in range(HT):
                    w2t = w2p.tile([128, FT, 128], BF16, tag="w2")
                    nc.sync.dma_start(
                        w2t, d_w2[l].rearrange("(kt kp) m -> kp kt m",
                                               kp=128)[:, :, m * 128:(m + 1) * 128])
                    ps = ps_proj.tile([128, S], F32, tag="proj")
                    for kt in range(FT):
                        nc.tensor.matmul(ps, w2t[:, kt, :], uT[:, kt, :S],
                                         start=(kt == 0), stop=(kt == FT - 1))
                    nc.scalar.activation(ps, ps, AF.Identity,
                                         bias=lc2[:, m:m + 1])
                    nc.vector.tensor_add(hT[g][:, m, :], hT[g][:, m, :], ps)

        # ---------------- output: h[:, 0, :] per graph ----------------
        for g in range(gpc):
            nc.sync.dma_start(d_hout[g].rearrange("t p -> p t"), hT[g][:, :, 0])

        # close pools in LIFO order: singles first, then named pools reversed
        for f in reversed(_frees):
            f()
        for p in (ps_o, ps_st, ps_proj, ps_misc, xp, stp, sqp, wk_, ep, bip,
                  lcp, w2p, w1p, wop, wvp, wkp, wqp, const):
            p.release()

    nc.compile()
    return nc


# ================= host side =================

def _bf16(a):
    return np.asarray(a, np.float32).astype(ml_dtypes.bfloat16)


def _pcol(v):
    """[..., H-like] -> [..., 128, ntiles] per-partition layout."""
    nt = v.shape[-1] // 128
    return np.ascontiguousarray(
        v.reshape(v.shape[:-1] + (nt, 128)).swapaxes(-1, -2)).astype(np.float32)


def prep_inputs(inp, n_layers=L, gpc=GPC):
    """Full inputs dict -> list of per-core in_maps."""
    x = np.asarray(inp["x"], np.float32)
    attn_bias = np.asarray(inp["attn_bias"], np.float32)
    gvd = np.asarray(inp["gvd"], np.float32)
    Wb = np.asarray(inp["Wbias"], np.float32)      # [L, BD, HEADS]
    bbias = np.asarray(inp["bbias"], np.float32)   # [L, HEADS]

    # xT padded (col 448 = 0)
    xT = np.zeros((B, 128, S), np.float32)
    xT[:, :, :N] = x.transpose(0, 2, 1)
    xT = _bf16(xT)

    gtT = _pcol(np.asarray(inp["graph_token"], np.float32)[0][None])[0]  # [128, HT]
    encW = _bf16(inp["enc_W"])
    encB = _pcol(np.asarray(inp["enc_b"], np.float32)[None])[0]

    wq = _bf16(np.asarray(inp["Wq"], np.float32) * SCALE)
    wk = _bf16(inp["Wk"])
    wv = _bf16(inp["Wv"])
    wo = _bf16(inp["Wo"])
    bq = _pcol(np.asarray(inp["bq"], np.float32) * SCALE)
    bk = _pcol(np.asarray(inp["bk"], np.float32))
    bo = _pcol(np.asarray(inp["bo"], np.float32))
    bvb = np.ascontiguousarray(np.broadcast_to(
        np.asarray(inp["bv"], np.float32)[:, None, :], (L, 128, H))).astype(np.float32)
    l1g = _pcol(np.asarray(inp["ln1_g"], np.float32))
    l1b = _pcol(np.asarray(inp["ln1_b"], np.float32))
    l2g = _pcol(np.asarray(inp["ln2_g"], np.float32))
    l2b = _pcol(np.asarray(inp["ln2_b"], np.float32))
    w1 = _bf16(inp["W1"])
    b1 = _pcol(np.asarray(inp["b1"], np.float32))
    w2 = _bf16(inp["W2"])
    b2 = _pcol(np.asarray(inp["b2"], np.float32))

    # ---- attention bias projection on host ----
    # gb[b, q, k, c]; bias[l, h, q, k] = sum_c gb[q, k, c] W[l, c, h] + bbias
    # device wants [g, l, h, kp, kc, q] (k = kc*128 + kp), pad k>=S with NEG
    Wall = Wb.transpose(1, 0, 2).reshape(BD, L * HEADS)   # [c, l*h]
    biasp = np.empty((B, L, HEADS, 128, 4, S), ml_dtypes.bfloat16)
    gvd0 = gvd[0]
    for b in range(B):
        gb = np.empty((S, S, BD), np.float32)
        gb[:N, :N] = attn_bias[b]
        gb[N, :N] = gvd0
        gb[:, N] = gvd0
        pr = gb.reshape(S * S, BD) @ Wall                 # [(q k), l*h]
        pr = pr.reshape(S, S, L, HEADS) + bbias[None, None]
        # -> [l, h, k, q], pad k to 512
        pr = pr.transpose(2, 3, 1, 0)
        prp = np.full((L, HEADS, SP, S), NEG, np.float32)
        prp[:, :, :S, :] = pr
        biasp[b] = _bf16(prp.reshape(L, HEADS, 4, 128, S).transpose(0, 1, 3, 2, 4))

    shared = dict(gtT=gtT, encW=encW, encB=encB, wq=wq, wk=wk, wv=wv, wo=wo,
                  bq=bq, bk=bk, bo=bo, bvb=bvb, l1g=l1g, l1b=l1b, l2g=l2g,
                  l2b=l2b, w1=w1, b1=b1, w2=w2, b2=b2)
    in_maps = []
    for core in range(B // gpc):
        m = dict(shared)
        m["xT"] = np.ascontiguousarray(xT[core * gpc:(core + 1) * gpc])
        m["biasp"] = np.ascontiguousarray(biasp[core * gpc:(core + 1) * gpc])
        in_maps.append(m)
    return in_maps


def finish_host(h0, inp):
    """h0: [B, H] final residual at node 0 (pre final-LN). -> [B, OUT] log_softmax."""
    fg = np.asarray(inp["fln_g"], np.float32)
    fb = np.asarray(inp["fln_b"], np.float32)
    oW = np.asarray(inp["out_W"], np.float32)
    ob = np.asarray(inp["out_b"], np.float32)
    m = h0.mean(-1, keepdims=True)
    v = np.square(h0 - m).mean(-1, keepdims=True)
    y = (h0 - m) / np.sqrt(v + EPS) * fg + fb
    logits = y @ oW + ob
    z = logits - logits.max(-1, keepdims=True)
    return (z - np.log(np.exp(z).sum(-1, keepdims=True))).astype(np.float32)


def kernel(**inputs):
    if "nc" not in _CACHE:
        _CACHE["nc"] = build_nc()
    nc = _CACHE["nc"]
    in_maps = prep_inputs(inputs)
    res = run_bass_kernel_spmd(nc, in_maps, core_ids=list(range(NCORES)))
    _CACHE["exec_time_ns"] = getattr(res, "exec_time_ns", None)
    _CACHE["res"] = res
    h0 = np.concatenate([r["hout"].reshape(GPC, H) for r in res.results], axis=0)
    return finish_host(h0, inputs)


if __name__ == "__main__":
    import reference
    inp = reference.setup_inputs()
    inp = {k: np.asarray(v) for k, v in inp.items()}
    out = kernel(**inp)
    exp = np.asarray(reference.reference(**inp))
    err = np.abs(out - exp).max() / np.abs(exp).max()
    print("Relative error:", err)

